# revision 37
# baseline (speedup 1.0000x reference)
"""Trainium2 Bass kernel for NeuralTensorLayer (order-1/2/3 polynomial layer).

    out[b,l] = bias[l] + sum_i X[b,i] W1[i,l]
             + sum_ij X[b,i] X[b,j] W2[i,j,l]
             + sum_ijk X[b,i] X[b,j] X[b,k] W3[i,j,k,l]

with B=32768, D=K=32, data-parallel over 8 NeuronCores (4096 rows each).

Strategy (per core):
  * Exploit (i,j) symmetry: only the 528 pairs i<=j are needed against
    host-symmetrized weights W3s[ij,k,l] = W3[i,j,k,l]+W3[j,i,k,l] (i<j),
    cutting the dominant matmul contraction from 1024 -> 528 (+32 X rows).
  * The pair operands Z^T[p,b] = X[b,i_p]X[b,j_p] arrive host-prepared in
    bf16 (the host also gathers/transposes every other operand; the device
    does all O(B*560*1056) contraction work plus the per-sample k-product
    and reductions).  Layout: ZB [s, 128, 4*512] holds chunks 0-3 side by
    side (one DMA per supertile; supertile 0 split so the PE starts early),
    Z4 [s, 48, 512] holds chunk 4 = 16 pairs + 32 order-1 X rows.
  * Per 128-row tile, one fp32-PSUM matmul group over two regions:
    A[b, l*32+k] (order-3 grid, 1024 cols = 2 psum banks, 3 bufs) and
    OB[b, t*32+l] (order-1+2 "out_low", 32 cols in a shared per-supertile
    psum tile).
  * Post, on a single-FIFO-friendly chain: ACT stages A to SBUF bf16 and
    copies the out_low slice (releasing the shared psum early); DVE
    multiplies by the X broadcast (2x), folds k 32->16 (aligned halves,
    2x), reduces 16->1 and adds out_low; bias added on host.  The very
    last tile reads PSUM directly to shorten the tail.
  * Input DMAs are split across the SYNC, GPSIMD and ACT queues (one
    sequencer alone bottlenecks at ~0.7us per DMA issue); supertile 0's
    weight and Z slices are woven across all three queues so the PE
    starts ~10us into the run.
"""

import numpy as np
import ml_dtypes
from contextlib import ExitStack

import concourse.bass as bass
import concourse.bacc as bacc
import concourse.tile as tile
from concourse import mybir
from concourse import bass_utils

BF16 = ml_dtypes.bfloat16

B, D, KOUT = 32768, 32, 32
NCORES = 8
BLOC = B // NCORES          # 4096 rows per core
P = 128                     # rows per tile
SUPER = 4                   # tiles per supertile
SP = SUPER * P              # 512
NSUPER = BLOC // SP         # 8
NPAIRS = D * (D + 1) // 2   # 528
NCA = KOUT * D              # 1024 order-3 psum columns (A region)
NCOL = NCA + KOUT           # 1056 weight columns (A grid + out_low)
KF = 16                     # folded k width (32 -> 16 via one aligned add)

PAIRS = [(i, j) for i in range(D) for j in range(i, D)]
I_P = np.array([p[0] for p in PAIRS], np.int32)
J_P = np.array([p[1] for p in PAIRS], np.int32)

F32 = mybir.dt.float32
BF = mybir.dt.bfloat16


# Drop redundant LDWEIGHTS from the BIR before walrus codegen: matmuls that
# share a stationary operand each carry their own Ldweights (walrus's
# ldw-opt pass is disabled/broken).  A load is elided when the previous PE
# weight-op in SCHEDULED order has a byte-identical weight AP and the load
# itself carries no semaphore waits/updates (so the PE weight registers
# provably still hold the same data and no sync edge is lost).
def _dedup_ldweights(bir_json: bytes) -> bytes:
    import json as _json

    d = _json.loads(bir_json)
    for fn in d.get("functions", []):
        for blk in fn.get("blocks", []):
            out = []
            last = None
            for i in blk.get("instructions", []):
                if i.get("engine") == "PE" and i.get("opcode") in ("Ldweights", "Matmult"):
                    w = i["ins"][-1] if i["opcode"] == "Matmult" else i["ins"][0]
                    key = (w.get("memref"), w.get("offset"), _json.dumps(w.get("ap")),
                           w.get("dtype"), _json.dumps(i.get("tile_position")),
                           _json.dumps(i.get("tile_size")), i.get("perf_mode"))
                    if i["opcode"] == "Ldweights":
                        si = i.get("sync_info") or {}
                        if (key == last and not si.get("on_wait")
                                and not si.get("on_update")):
                            continue
                        last = key
                    else:
                        # a Matmult's weight ref mirrors the loaded state
                        # (self-loading or not), so it may refresh `last`
                        last = key
                elif i.get("engine") == "PE":
                    last = None  # unknown PE op: invalidate weight-reuse state
                out.append(i)
            blk["instructions"] = out
    return _json.dumps(d).encode()


if not getattr(bass_utils, "_ldw_dedup_patched", False):
    _orig_compile_bir_kernel = bass_utils.compile_bir_kernel

    def _compile_bir_kernel_dedup(bir_json, tmpdir, neff_name="file.neff"):
        return _orig_compile_bir_kernel(_dedup_ldweights(bir_json), tmpdir, neff_name)

    bass_utils.compile_bir_kernel = _compile_bir_kernel_dedup
    import concourse.bass2jax as _b2j

    _b2j.compile_bir_kernel = _compile_bir_kernel_dedup
    bass_utils._ldw_dedup_patched = True


def _pack_weights(W1, W2, W3):
    W1 = np.asarray(W1, np.float64)
    W2 = np.asarray(W2, np.float64)
    W3 = np.asarray(W3, np.float64)
    Wcat = np.zeros((5, 128, NCOL), np.float64)
    for p, (i, j) in enumerate(PAIRS):
        c, pp = divmod(p, 128)
        if i < j:
            w3 = W3[i, j] + W3[j, i]   # [k, l]
            w2 = W2[i, j] + W2[j, i]   # [l]
        else:
            w3 = W3[i, i]
            w2 = W2[i, i]
        Wcat[c, pp, :NCA] = w3.T.reshape(-1)   # col l*32+k
        Wcat[c, pp, NCA:] = w2                 # out_low columns
    for dd in range(D):                # order-1: X rows in chunk 4
        Wcat[4, 16 + dd, NCA:] = W1[dd]
    return Wcat.astype(np.float32).astype(BF16)


def _build_module():
    nc = bacc.Bacc("TRN2", target_bir_lowering=False, debug=False,
                   enable_asserts=False)
    XBd = nc.dram_tensor("XBB", [NSUPER, P, SUPER * D], BF, kind="ExternalInput").ap()
    ZBd = nc.dram_tensor("ZB", [NSUPER, 128, 4 * SP], BF, kind="ExternalInput").ap()
    Z4d = nc.dram_tensor("Z4", [NSUPER, 48, SP], BF, kind="ExternalInput").ap()
    WCd = nc.dram_tensor("WCAT", [5, 128, NCOL], BF, kind="ExternalInput").ap()
    OUTd = nc.dram_tensor("OUT", [BLOC, KOUT], F32, kind="ExternalOutput").ap()

    with ExitStack() as ctx:
        tc = ctx.enter_context(tile.TileContext(nc))
        consts = ctx.enter_context(tc.tile_pool(name="consts", bufs=1))
        xbpool = ctx.enter_context(tc.tile_pool(name="xbpool", bufs=3))
        zpool = ctx.enter_context(tc.tile_pool(name="zpool", bufs=3))
        spool = ctx.enter_context(tc.tile_pool(name="spool", bufs=4))
        upool = ctx.enter_context(tc.tile_pool(name="upool", bufs=4))
        fpool = ctx.enter_context(tc.tile_pool(name="fpool", bufs=4))
        tpool = ctx.enter_context(tc.tile_pool(name="tpool", bufs=10))
        opool = ctx.enter_context(tc.tile_pool(name="opool", bufs=16))
        obsb = ctx.enter_context(tc.tile_pool(name="obsb", bufs=8))
        t3ps = ctx.enter_context(tc.tile_pool(name="t3ps", bufs=3, space="PSUM"))
        obps = ctx.enter_context(tc.tile_pool(name="obps", bufs=2, space="PSUM"))

        w_sb = [consts.tile([128, NCOL], BF, tag=f"w_{c}", name=f"w{c}")
                for c in range(5)]

        def build_dma(s):
            """Issue all input DMAs for supertile s."""
            zb = zpool.tile([128, 4 * SP], BF, tag="zb")
            z4 = zpool.tile([48, SP], BF, tag="z4")
            xbb = xbpool.tile([P, SUPER * D], BF, tag="xbb")
            if s == 0:
                # Weave the weight loads and the tile-0 column slices across
                # all three DMA queues so the PE can start chunk c at
                # ~(c+1) transfer-times into the run.
                def zslice(c, a, b, eng):
                    eng.dma_start(out=zb[:, c * SP + a:c * SP + b],
                                  in_=ZBd[0, :, c * SP + a:c * SP + b])
                # first weight in column halves: the first matmul only needs
                # cols 0:512, so it can start one transfer-time earlier
                nc.scalar.dma_start(out=w_sb[0][:, :512], in_=WCd[0, :, :512])
                zslice(0, 0, P, nc.sync)
                zslice(1, 0, P, nc.gpsimd)
                nc.scalar.dma_start(out=w_sb[0][:, 512:], in_=WCd[0, :, 512:])
                nc.sync.dma_start(out=w_sb[1], in_=WCd[1])
                nc.gpsimd.dma_start(out=w_sb[2], in_=WCd[2])
                zslice(2, 0, P, nc.sync)
                zslice(3, 0, P, nc.gpsimd)
                nc.scalar.dma_start(out=w_sb[3], in_=WCd[3])
                nc.scalar.dma_start(out=w_sb[4], in_=WCd[4])
                nc.gpsimd.dma_start(out=z4[:, :P], in_=Z4d[s, :, :P])
                # tile-1 columns next, then the rest: a single big "rest"
                # transfer makes tile 1 stall ~6us and the hiccup echoes
                # down the psum->stage recycling chain for ~10 tiles
                zslice(0, P, 2 * P, nc.sync)
                zslice(1, P, 2 * P, nc.gpsimd)
                zslice(2, P, 2 * P, nc.sync)
                zslice(3, P, 2 * P, nc.gpsimd)
                nc.gpsimd.dma_start(out=z4[:, P:], in_=Z4d[s, :, P:])
                nc.gpsimd.dma_start(out=xbb, in_=XBd[s])
                zslice(0, 2 * P, SP, nc.sync)
                zslice(1, 2 * P, SP, nc.gpsimd)
                zslice(2, 2 * P, SP, nc.sync)
                zslice(3, 2 * P, SP, nc.gpsimd)
            else:
                # keep the big input transfer off the sync queue: the OUT
                # DMAs live there, and an output stuck behind a 2.7us zb
                # transfer delays osb recycling and paces the DVE chain
                nc.gpsimd.dma_start(out=z4, in_=Z4d[s])
                nc.gpsimd.dma_start(out=xbb, in_=XBd[s])
                nc.gpsimd.dma_start(out=zb, in_=ZBd[s])
            return xbb, zb, z4

        NSPLITS_A = ((0, 512), (512, 1024))

        def post(t3, ob, xbb, row0, t, direct=False):
            """Stage/multiply/fold/reduce one tile's psum into OUT."""
            if direct:
                src = t3            # read psum directly (tail latency path)
                obv = ob[:, t * KOUT:(t + 1) * KOUT]
            else:
                staged = spool.tile([P, NCA], BF, tag="staged")
                # two half-column copies: each psum bank region is released
                # as soon as its own copy completes, halving the recycling
                # latency the next tile's first matmul can stall on
                nc.scalar.copy(out=staged[:, :512], in_=t3[:, :512])
                nc.scalar.copy(out=staged[:, 512:], in_=t3[:, 512:])
                src = staged
                # small ACT copy releases the shared ob psum tile early, so
                # the next supertile's first matmul never waits on the
                # DVE post chain
                obs = obsb.tile([P, KOUT], F32, tag="obs")
                nc.scalar.copy(out=obs, in_=ob[:, t * KOUT:(t + 1) * KOUT])
                obv = obs
            u = upool.tile([P, NCA], BF, tag="u")
            xk = xbb[:, t * D:(t + 1) * D].unsqueeze(1).broadcast_to([P, KOUT, D])
            nc.vector.tensor_mul(
                u[:, :].rearrange("p (l k) -> p l k", k=D),
                src[:, :].rearrange("p (l k) -> p l k", k=D),
                xk,
            )
            # fold k 32 -> 16 (aligned halves, 2x on DVE), then reduce + out_low
            f = fpool.tile([P, KOUT * KF], BF, tag="f")
            ur = u[:, :].rearrange("p (l k) -> p l k", k=D)
            nc.vector.tensor_add(
                f[:, :].rearrange("p (l k) -> p l k", k=KF),
                ur[:, :, 0:KF],
                ur[:, :, KF:D],
            )
            red = tpool.tile([P, KOUT], F32, tag="red")
            nc.vector.reduce_sum(
                out=red, in_=f[:, :].rearrange("p (l k) -> p l k", k=KF),
                axis=mybir.AxisListType.X,
            )
            osb = opool.tile([P, KOUT], F32, tag="osb")
            nc.vector.tensor_add(osb, red, obv)
            nc.sync.dma_start(out=OUTd[row0 + t * P: row0 + (t + 1) * P, :],
                              in_=osb)

        state = build_dma(0)
        for s in range(NSUPER):
            xbb, zb, z4 = state
            if s + 1 < NSUPER:
                nstate = build_dma(s + 1)
            row0 = s * SP
            ob = obps.tile([P, SUPER * KOUT], F32, tag="ob")
            for t in range(SUPER):
                t3 = t3ps.tile([P, NCA], F32, tag="t3")
                for c in range(5):
                    if c < 4:
                        zc = zb[:, c * SP + t * P: c * SP + (t + 1) * P]
                        wc = w_sb[c]
                    else:
                        zc = z4[:, t * P: (t + 1) * P]
                        wc = w_sb[4][:48]
                    for n0, n1 in NSPLITS_A:
                        nc.tensor.matmul(t3[:, n0:n1], zc, wc[:, n0:n1],
                                         start=(c == 0), stop=(c == 4))
                    nc.tensor.matmul(ob[:, t * KOUT:(t + 1) * KOUT], zc,
                                     wc[:, NCA:],
                                     start=(c == 0), stop=(c == 4))
                last_tile = (s == NSUPER - 1 and t == SUPER - 1)
                post(t3, ob, xbb, row0, t, direct=last_tile)
            if s + 1 < NSUPER:
                state = nstate
    nc.compile()
    return nc


_CACHE = {}


def _get_module():
    if "nc" not in _CACHE:
        _CACHE["nc"] = _build_module()
    return _CACHE["nc"]


def kernel(X, W1, W2, W3, bias):
    X = np.ascontiguousarray(np.asarray(X, np.float32))
    bias = np.asarray(bias, np.float32)
    Wcat = _pack_weights(W1, W2, W3)

    nc = _get_module()
    Xb = X.astype(BF16)                      # [B, D] bf16 (single rounding point)
    Xf = Xb.astype(np.float32)
    XbT = np.ascontiguousarray(Xb.T)         # [D, B] bf16
    Z = (Xf[:, I_P] * Xf[:, J_P]).astype(BF16).T  # [528, B] pair products
    Z = np.ascontiguousarray(Z)
    # chunks 0-3 batched per supertile: [ncores, NSUPER, 128, 4*SP]
    ZB = np.ascontiguousarray(
        Z[:512].reshape(4, 128, NCORES, NSUPER, SP)
        .transpose(2, 3, 1, 0, 4).reshape(NCORES, NSUPER, 128, 4 * SP))
    # chunk 4: 16 pair rows + 32 X^T rows (order-1)
    Z4 = np.concatenate([Z[512:528], XbT], axis=0)  # [48, B]
    Z4 = np.ascontiguousarray(
        Z4.reshape(48, NCORES, NSUPER, SP).transpose(1, 2, 0, 3))
    XBB = np.ascontiguousarray(
        Xb.reshape(NCORES, NSUPER, SUPER, P, D)
        .transpose(0, 1, 3, 2, 4).reshape(NCORES, NSUPER, P, SUPER * D))
    in_maps = [
        {"XBB": XBB[c], "ZB": ZB[c], "Z4": Z4[c], "WCAT": Wcat}
        for c in range(NCORES)
    ]
    res = bass_utils.run_bass_kernel_spmd(nc, in_maps, core_ids=list(range(NCORES)))
    _CACHE["last_results"] = res
    out = np.concatenate([np.asarray(res.results[c]["OUT"]) for c in range(NCORES)], 0)
    return (out + bias.reshape(1, KOUT)).astype(np.float32)


# revision 41
# speedup vs baseline: 1.0284x; 1.0284x over previous
"""Trainium2 Bass kernel for NeuralTensorLayer (order-1/2/3 polynomial layer).

    out[b,l] = bias[l] + sum_i X[b,i] W1[i,l]
             + sum_ij X[b,i] X[b,j] W2[i,j,l]
             + sum_ijk X[b,i] X[b,j] X[b,k] W3[i,j,k,l]

with B=32768, D=K=32, data-parallel over 8 NeuronCores (4096 rows each).

Strategy (per core):
  * Exploit (i,j) symmetry: only the 528 pairs i<=j are needed against
    host-symmetrized weights W3s[ij,k,l] = W3[i,j,k,l]+W3[j,i,k,l] (i<j),
    cutting the dominant matmul contraction from 1024 -> 528 (+32 X rows).
  * The pair operands Z^T[p,b] = X[b,i_p]X[b,j_p] arrive host-prepared in
    bf16 (the host also gathers/transposes every other operand; the device
    does all O(B*560*1056) contraction work plus the per-sample k-product
    and reductions).  Layout: ZB [s, 128, 4*512] holds chunks 0-3 side by
    side (one DMA per supertile; supertile 0 split so the PE starts early),
    Z4 [s, 48, 512] holds chunk 4 = 16 pairs + 32 order-1 X rows.
  * Per 128-row tile, one fp32-PSUM matmul group over two regions:
    A[b, l*32+k] (order-3 grid, 1024 cols = 2 psum banks, 3 bufs) and
    OB[b, t*32+l] (order-1+2 "out_low", 32 cols in a shared per-supertile
    psum tile).
  * Post, on a single-FIFO-friendly chain: ACT stages A to SBUF bf16 and
    copies the out_low slice (releasing the shared psum early); DVE
    multiplies by the X broadcast (2x), folds k 32->16 (aligned halves,
    2x), reduces 16->1 and adds out_low; bias added on host.  The very
    last tile reads PSUM directly to shorten the tail.
  * Input DMAs are split across the SYNC, GPSIMD and ACT queues (one
    sequencer alone bottlenecks at ~0.7us per DMA issue); supertile 0's
    weight and Z slices are woven across all three queues so the PE
    starts ~10us into the run.
"""

import numpy as np
import ml_dtypes
from contextlib import ExitStack

import concourse.bass as bass
import concourse.bacc as bacc
import concourse.tile as tile
from concourse import mybir
from concourse import bass_utils

BF16 = ml_dtypes.bfloat16

B, D, KOUT = 32768, 32, 32
NCORES = 8
BLOC = B // NCORES          # 4096 rows per core
P = 128                     # rows per tile
SUPER = 8                   # tiles per supertile
SP = SUPER * P              # 512
NSUPER = BLOC // SP         # 8
NPAIRS = D * (D + 1) // 2   # 528
NCA = KOUT * D              # 1024 order-3 psum columns (A region)
NCOL = NCA + KOUT           # 1056 weight columns (A grid + out_low)
KF = 16                     # folded k width (32 -> 16 via one aligned add)

PAIRS = [(i, j) for i in range(D) for j in range(i, D)]
I_P = np.array([p[0] for p in PAIRS], np.int32)
J_P = np.array([p[1] for p in PAIRS], np.int32)

F32 = mybir.dt.float32
BF = mybir.dt.bfloat16


# Drop redundant LDWEIGHTS from the BIR before walrus codegen: matmuls that
# share a stationary operand each carry their own Ldweights (walrus's
# ldw-opt pass is disabled/broken).  A load is elided when the previous PE
# weight-op in SCHEDULED order has a byte-identical weight AP and the load
# itself carries no semaphore waits/updates (so the PE weight registers
# provably still hold the same data and no sync edge is lost).
def _dedup_ldweights(bir_json: bytes) -> bytes:
    import json as _json

    d = _json.loads(bir_json)
    for fn in d.get("functions", []):
        for blk in fn.get("blocks", []):
            out = []
            last = None
            for i in blk.get("instructions", []):
                if i.get("engine") == "PE" and i.get("opcode") in ("Ldweights", "Matmult"):
                    w = i["ins"][-1] if i["opcode"] == "Matmult" else i["ins"][0]
                    key = (w.get("memref"), w.get("offset"), _json.dumps(w.get("ap")),
                           w.get("dtype"), _json.dumps(i.get("tile_position")),
                           _json.dumps(i.get("tile_size")), i.get("perf_mode"))
                    if i["opcode"] == "Ldweights":
                        si = i.get("sync_info") or {}
                        if (key == last and not si.get("on_wait")
                                and not si.get("on_update")):
                            continue
                        last = key
                    else:
                        # a Matmult's weight ref mirrors the loaded state
                        # (self-loading or not), so it may refresh `last`
                        last = key
                elif i.get("engine") == "PE":
                    last = None  # unknown PE op: invalidate weight-reuse state
                out.append(i)
            blk["instructions"] = out
    return _json.dumps(d).encode()


if not getattr(bass_utils, "_ldw_dedup_patched", False):
    _orig_compile_bir_kernel = bass_utils.compile_bir_kernel

    def _compile_bir_kernel_dedup(bir_json, tmpdir, neff_name="file.neff"):
        return _orig_compile_bir_kernel(_dedup_ldweights(bir_json), tmpdir, neff_name)

    bass_utils.compile_bir_kernel = _compile_bir_kernel_dedup
    import concourse.bass2jax as _b2j

    _b2j.compile_bir_kernel = _compile_bir_kernel_dedup
    bass_utils._ldw_dedup_patched = True


def _pack_weights(W1, W2, W3):
    W1 = np.asarray(W1, np.float64)
    W2 = np.asarray(W2, np.float64)
    W3 = np.asarray(W3, np.float64)
    Wcat = np.zeros((5, 128, NCOL), np.float64)
    for p, (i, j) in enumerate(PAIRS):
        c, pp = divmod(p, 128)
        if i < j:
            w3 = W3[i, j] + W3[j, i]   # [k, l]
            w2 = W2[i, j] + W2[j, i]   # [l]
        else:
            w3 = W3[i, i]
            w2 = W2[i, i]
        Wcat[c, pp, :NCA] = w3.T.reshape(-1)   # col l*32+k
        Wcat[c, pp, NCA:] = w2                 # out_low columns
    for dd in range(D):                # order-1: X rows in chunk 4
        Wcat[4, 16 + dd, NCA:] = W1[dd]
    return Wcat.astype(np.float32).astype(BF16)


def _build_module():
    nc = bacc.Bacc("TRN2", target_bir_lowering=False, debug=False,
                   enable_asserts=False)
    XBd = nc.dram_tensor("XBB", [NSUPER, P, SUPER * D], BF, kind="ExternalInput").ap()
    ZBd = nc.dram_tensor("ZB", [NSUPER, 128, 4 * SP], BF, kind="ExternalInput").ap()
    Z4d = nc.dram_tensor("Z4", [NSUPER, 48, SP], BF, kind="ExternalInput").ap()
    WCd = nc.dram_tensor("WCAT", [5, 128, NCOL], BF, kind="ExternalInput").ap()
    OUTd = nc.dram_tensor("OUT", [BLOC, KOUT], F32, kind="ExternalOutput").ap()

    with ExitStack() as ctx:
        tc = ctx.enter_context(tile.TileContext(nc))
        consts = ctx.enter_context(tc.tile_pool(name="consts", bufs=1))
        xbpool = ctx.enter_context(tc.tile_pool(name="xbpool", bufs=3))
        zpool = ctx.enter_context(tc.tile_pool(name="zpool", bufs=3))
        spool = ctx.enter_context(tc.tile_pool(name="spool", bufs=4))
        upool = ctx.enter_context(tc.tile_pool(name="upool", bufs=4))
        fpool = ctx.enter_context(tc.tile_pool(name="fpool", bufs=4))
        tpool = ctx.enter_context(tc.tile_pool(name="tpool", bufs=10))
        opool = ctx.enter_context(tc.tile_pool(name="opool", bufs=16))
        obsb = ctx.enter_context(tc.tile_pool(name="obsb", bufs=8))
        t3ps = ctx.enter_context(tc.tile_pool(name="t3ps", bufs=3, space="PSUM"))
        obps = ctx.enter_context(tc.tile_pool(name="obps", bufs=2, space="PSUM"))

        w_sb = [consts.tile([128, NCOL], BF, tag=f"w_{c}", name=f"w{c}")
                for c in range(5)]

        def build_dma(s):
            """Issue all input DMAs for supertile s."""
            zb = zpool.tile([128, 4 * SP], BF, tag="zb")
            z4 = zpool.tile([48, SP], BF, tag="z4")
            xbb = xbpool.tile([P, SUPER * D], BF, tag="xbb")
            if s == 0:
                # Weave the weight loads and the tile-0 column slices across
                # all three DMA queues so the PE can start chunk c at
                # ~(c+1) transfer-times into the run.
                def zslice(c, a, b, eng):
                    eng.dma_start(out=zb[:, c * SP + a:c * SP + b],
                                  in_=ZBd[0, :, c * SP + a:c * SP + b])
                # first weight in column halves: the first matmul only needs
                # cols 0:512, so it can start one transfer-time earlier
                nc.scalar.dma_start(out=w_sb[0][:, :512], in_=WCd[0, :, :512])
                zslice(0, 0, P, nc.sync)
                zslice(1, 0, P, nc.gpsimd)
                nc.scalar.dma_start(out=w_sb[0][:, 512:], in_=WCd[0, :, 512:])
                nc.sync.dma_start(out=w_sb[1], in_=WCd[1])
                nc.gpsimd.dma_start(out=w_sb[2], in_=WCd[2])
                zslice(2, 0, P, nc.sync)
                zslice(3, 0, P, nc.gpsimd)
                nc.scalar.dma_start(out=w_sb[3], in_=WCd[3])
                nc.scalar.dma_start(out=w_sb[4], in_=WCd[4])
                nc.gpsimd.dma_start(out=z4[:, :P], in_=Z4d[s, :, :P])
                # tile-1 columns next, then the rest: a single big "rest"
                # transfer makes tile 1 stall ~6us and the hiccup echoes
                # down the psum->stage recycling chain for ~10 tiles
                zslice(0, P, 2 * P, nc.sync)
                zslice(1, P, 2 * P, nc.gpsimd)
                zslice(2, P, 2 * P, nc.sync)
                zslice(3, P, 2 * P, nc.gpsimd)
                nc.gpsimd.dma_start(out=z4[:, P:], in_=Z4d[s, :, P:])
                nc.gpsimd.dma_start(out=xbb, in_=XBd[s])
                zslice(0, 2 * P, SP, nc.sync)
                zslice(1, 2 * P, SP, nc.gpsimd)
                zslice(2, 2 * P, SP, nc.sync)
                zslice(3, 2 * P, SP, nc.gpsimd)
            else:
                # keep the big input transfer off the sync queue: the OUT
                # DMAs live there, and an output stuck behind a 2.7us zb
                # transfer delays osb recycling and paces the DVE chain
                nc.gpsimd.dma_start(out=z4, in_=Z4d[s])
                nc.gpsimd.dma_start(out=xbb, in_=XBd[s])
                nc.gpsimd.dma_start(out=zb, in_=ZBd[s])
            return xbb, zb, z4

        NSPLITS_A = ((0, 512), (512, 1024))

        def post(t3, ob, xbb, row0, t, direct=False):
            """Stage/multiply/fold/reduce one tile's psum into OUT."""
            if direct:
                src = t3            # read psum directly (tail latency path)
                obv = ob[:, t * KOUT:(t + 1) * KOUT]
            else:
                staged = spool.tile([P, NCA], BF, tag="staged")
                nc.scalar.copy(out=staged, in_=t3)
                src = staged
                # small ACT copy releases the shared ob psum tile early, so
                # the next supertile's first matmul never waits on the
                # DVE post chain
                obs = obsb.tile([P, KOUT], F32, tag="obs")
                nc.scalar.copy(out=obs, in_=ob[:, t * KOUT:(t + 1) * KOUT])
                obv = obs
            u = upool.tile([P, NCA], BF, tag="u")
            xk = xbb[:, t * D:(t + 1) * D].unsqueeze(1).broadcast_to([P, KOUT, D])
            nc.vector.tensor_mul(
                u[:, :].rearrange("p (l k) -> p l k", k=D),
                src[:, :].rearrange("p (l k) -> p l k", k=D),
                xk,
            )
            # fold k 32 -> 16 (aligned halves, 2x on DVE), then reduce + out_low
            f = fpool.tile([P, KOUT * KF], BF, tag="f")
            ur = u[:, :].rearrange("p (l k) -> p l k", k=D)
            nc.vector.tensor_add(
                f[:, :].rearrange("p (l k) -> p l k", k=KF),
                ur[:, :, 0:KF],
                ur[:, :, KF:D],
            )
            red = tpool.tile([P, KOUT], F32, tag="red")
            nc.vector.reduce_sum(
                out=red, in_=f[:, :].rearrange("p (l k) -> p l k", k=KF),
                axis=mybir.AxisListType.X,
            )
            osb = opool.tile([P, KOUT], F32, tag="osb")
            nc.vector.tensor_add(osb, red, obv)
            nc.sync.dma_start(out=OUTd[row0 + t * P: row0 + (t + 1) * P, :],
                              in_=osb)

        state = build_dma(0)
        for s in range(NSUPER):
            xbb, zb, z4 = state
            if s + 1 < NSUPER:
                nstate = build_dma(s + 1)
            row0 = s * SP
            ob = obps.tile([P, SUPER * KOUT], F32, tag="ob")
            for t in range(SUPER):
                t3 = t3ps.tile([P, NCA], F32, tag="t3")
                for c in range(5):
                    if c < 4:
                        zc = zb[:, c * SP + t * P: c * SP + (t + 1) * P]
                        wc = w_sb[c]
                    else:
                        zc = z4[:, t * P: (t + 1) * P]
                        wc = w_sb[4][:48]
                    for n0, n1 in NSPLITS_A:
                        nc.tensor.matmul(t3[:, n0:n1], zc, wc[:, n0:n1],
                                         start=(c == 0), stop=(c == 4))
                    nc.tensor.matmul(ob[:, t * KOUT:(t + 1) * KOUT], zc,
                                     wc[:, NCA:],
                                     start=(c == 0), stop=(c == 4))
                last_tile = (s == NSUPER - 1 and t == SUPER - 1)
                post(t3, ob, xbb, row0, t, direct=last_tile)
            if s + 1 < NSUPER:
                state = nstate
    nc.compile()
    return nc


_CACHE = {}


def _get_module():
    if "nc" not in _CACHE:
        _CACHE["nc"] = _build_module()
    return _CACHE["nc"]


def kernel(X, W1, W2, W3, bias):
    X = np.ascontiguousarray(np.asarray(X, np.float32))
    bias = np.asarray(bias, np.float32)
    Wcat = _pack_weights(W1, W2, W3)

    nc = _get_module()
    Xb = X.astype(BF16)                      # [B, D] bf16 (single rounding point)
    Xf = Xb.astype(np.float32)
    XbT = np.ascontiguousarray(Xb.T)         # [D, B] bf16
    Z = (Xf[:, I_P] * Xf[:, J_P]).astype(BF16).T  # [528, B] pair products
    Z = np.ascontiguousarray(Z)
    # chunks 0-3 batched per supertile: [ncores, NSUPER, 128, 4*SP]
    ZB = np.ascontiguousarray(
        Z[:512].reshape(4, 128, NCORES, NSUPER, SP)
        .transpose(2, 3, 1, 0, 4).reshape(NCORES, NSUPER, 128, 4 * SP))
    # chunk 4: 16 pair rows + 32 X^T rows (order-1)
    Z4 = np.concatenate([Z[512:528], XbT], axis=0)  # [48, B]
    Z4 = np.ascontiguousarray(
        Z4.reshape(48, NCORES, NSUPER, SP).transpose(1, 2, 0, 3))
    XBB = np.ascontiguousarray(
        Xb.reshape(NCORES, NSUPER, SUPER, P, D)
        .transpose(0, 1, 3, 2, 4).reshape(NCORES, NSUPER, P, SUPER * D))
    in_maps = [
        {"XBB": XBB[c], "ZB": ZB[c], "Z4": Z4[c], "WCAT": Wcat}
        for c in range(NCORES)
    ]
    res = bass_utils.run_bass_kernel_spmd(nc, in_maps, core_ids=list(range(NCORES)))
    _CACHE["last_results"] = res
    out = np.concatenate([np.asarray(res.results[c]["OUT"]) for c in range(NCORES)], 0)
    return (out + bias.reshape(1, KOUT)).astype(np.float32)
